# revision 4
# baseline (speedup 1.0000x reference)
"""AdapterGNN distributed Trainium2 kernel (8 NeuronCores, Bass/Tile).

out = norm_dst * segsum_dst( (X*norm_src) @ Wd + norm_src*bd )[src] @ (Wg@Wu) + (bg@Wu+bu)

Sharding: nodes are split across 8 cores; edges are partitioned by dst owner.
The down-projected node features h (fp16) are AllGathered so every per-edge
gather h[src] is core-local. Gathers run as one indirect DMA per slot column
(the HW consumes one offset per partition and reads the out free-size
contiguously). The h table exceeds the indirect-DMA descriptor offset range
(2^24 bytes), so each column addresses one of two overlapping table regions
selected via element_offset; overlap edges flex to either side so the column
count stays near the unsplit max in-degree. Agg slots are sorted by
(deg, low-count, high-count) — decoupled from the table-row order — to
tighten per-window column maxima. The segment-sum is a strided DVE reduction
over slot columns; norm_dst scaling, the fused (Wg@Wu) up-projection and the
bias ride the same per-window DVE/PE pipeline.

Self-contained: requires only numpy + concourse (+ TRN2 cores via axon).
"""

import numpy as np

import concourse.bacc as bacc
import concourse.mybir as mybir
import concourse.tile as tile
from concourse.bass import IndirectOffsetOnAxis
from concourse.bass_utils import run_bass_kernel_spmd
from concourse.masks import make_identity

F32 = mybir.dt.float32
F16 = mybir.dt.float16
I32 = mybir.dt.int32

P = 128  # partitions


class Cfg:
    def __init__(self, n_nodes, n_edges, in_dim, out_dim, n_cores=8, wpg=4):
        self.N = n_nodes
        self.E = n_edges
        self.IN = in_dim          # 768
        self.OUT = out_dim        # 128 (must be 128)
        self.C = n_cores
        assert out_dim == P
        self.NpReal = (n_nodes + n_cores - 1) // n_cores   # real nodes per core
        self.W = (self.NpReal + P - 1) // P                # windows per core
        self.Np = self.W * P                               # padded nodes/core
        self.KC = in_dim // P                              # full K chunks (6)
        assert in_dim % P == 0
        self.KIN = self.KC + 1                             # +1 chunk for (norm,bias) row
        self.sentinel = n_cores * self.Np - 1              # guaranteed-zero h row
        self.wpg = wpg                                     # windows per gather group


def host_prep(cfg, features, Wd, bd, Wg, bg, Wu, bu, src, dst, force_split=False):
    """Returns (in_maps, node_core, node_slot, k_prof)."""
    C, N, Np, W = cfg.C, cfg.N, cfg.Np, cfg.W
    src = np.asarray(src).astype(np.int64)
    dst = np.asarray(dst).astype(np.int64)
    features = np.asarray(features, dtype=np.float32)

    out_deg = np.bincount(src, minlength=N)
    in_deg = np.bincount(dst, minlength=N)
    norm_src = 1.0 / np.sqrt(np.maximum(out_deg, 1.0))
    norm_dst = 1.0 / np.sqrt(np.maximum(in_deg, 1.0))

    # node -> core; TABLE slot = id order (fixed first, so edge regions are
    # known before choosing agg slots)
    node_core = np.minimum(np.arange(N) // cfg.NpReal, C - 1)
    slot_tab = np.arange(N, dtype=np.int64) - node_core * cfg.NpReal
    table_row = node_core * Np + slot_tab  # h table row of each node

    # The indirect-DMA descriptor offset is < 2^24 bytes, so a gather
    # instruction can only address LIMIT_ROWS of the fp16 table. Use two
    # OVERLAPPING regions: LO = rows [0, LIMIT_ROWS), HI = rows
    # [hi_base, n_rows) with hi_base = n_rows - LIMIT_ROWS (via
    # element_offset). Edges in the overlap flex to either side, so the
    # column count stays ~ the unsplit max degree.
    n_rows = C * Np
    LIMIT_ROWS = (1 << 24) // (cfg.OUT * 2)        # 65536 for OUT=128 fp16
    if force_split and LIMIT_ROWS >= n_rows:
        LIMIT_ROWS = max(Np, (n_rows * 2) // 3)
    hi_base = max(0, n_rows - LIMIT_ROWS)
    split = hi_base > 0
    erow = table_row[src]                          # table row of each edge's src
    e_lowonly = erow < hi_base
    e_highonly = erow >= LIMIT_ROWS

    # AGG slot (window position) per node: sorted by (deg, low-only count,
    # high-only count) desc so per-window maxima of all three are tight.
    low_cnt = np.bincount(dst[e_lowonly], minlength=N)
    high_cnt = np.bincount(dst[e_highonly], minlength=N)
    node_slot = np.empty(N, dtype=np.int64)
    core_nodes = []  # per-core node ids in agg-slot order
    for c in range(C):
        ns = np.where(node_core == c)[0]
        order = np.lexsort((-high_cnt[ns], -low_cnt[ns], -in_deg[ns]))
        ns_sorted = ns[order]
        node_slot[ns_sorted] = np.arange(len(ns_sorted))
        core_nodes.append(ns_sorted)
    # kA/kB profiles (shared across cores): kA >= max low-only count,
    # kB >= max(high-only count, deg - kA)
    kA = np.zeros(W, dtype=np.int64)
    kHo = np.zeros(W, dtype=np.int64)
    kDeg = np.zeros(W, dtype=np.int64)
    for c in range(C):
        em = np.where(node_core[dst] == c)[0]
        ds = node_slot[dst[em]]
        degT = np.bincount(ds, minlength=Np).reshape(W, P)
        degL = np.bincount(ds[e_lowonly[em]], minlength=Np).reshape(W, P)
        degH = np.bincount(ds[e_highonly[em]], minlength=Np).reshape(W, P)
        kA = np.maximum(kA, degL.max(axis=1))
        kHo = np.maximum(kHo, degH.max(axis=1))
        kDeg = np.maximum(kDeg, degT.max(axis=1))
    if split:
        kA = np.maximum(kA, 1)
        kB = np.maximum(np.maximum(kHo, kDeg - kA), 1)
    else:
        kA = np.maximum(kDeg, 1)
        kB = np.zeros(W, dtype=np.int64)
    k_prof = kA + kB
    offs = np.concatenate([[0], np.cumsum(k_prof)])
    offsA = offs[:-1]              # low-region block start per window
    offsB = offs[:-1] + kA         # high-region block start per window
    T = int(offs[-1])

    # build per-core inputs
    in_maps = []
    Wgu = (Wg.astype(np.float64) @ Wu.astype(np.float64)).astype(np.float32)
    bu2 = (bg.astype(np.float64) @ Wu.astype(np.float64) + bu).astype(np.float32)
    wd_h = np.zeros((P, cfg.KIN * cfg.OUT), dtype=np.float16)
    for cc in range(cfg.KC):
        wd_h[:, cc * cfg.OUT : (cc + 1) * cfg.OUT] = Wd[cc * P : (cc + 1) * P, :]
    wd_h[0, cfg.KC * cfg.OUT : (cfg.KC + 1) * cfg.OUT] = bd
    wgu_h = Wgu.astype(np.float16)
    bu2_h = np.tile(bu2[None, :], (P, 1)).astype(np.float32)

    edge_core = node_core[dst]
    for c in range(C):
        ns = core_nodes[c]
        n_real = len(ns)
        # xa is keyed by TABLE slot (= id order within the core)
        nt_ids = np.where(node_core == np.int64(c))[0]
        xa = np.zeros((P, cfg.KIN * Np), dtype=np.float16)
        xs = (features[nt_ids, :] * norm_src[nt_ids, None]).astype(np.float16)
        for cc in range(cfg.KC):
            xa[:, cc * Np : cc * Np + n_real] = xs[:, cc * P : (cc + 1) * P].T
        xa[0, cfg.KC * Np : cfg.KC * Np + n_real] = norm_src[nt_ids].astype(np.float16)

        # edges of this core: sort by (dst slot, region-preference). Each dst
        # fills its LO block with its first L = min(kA, #low-only+#overlap)
        # edges (low-only lead, then overlap); the tail spills to its HI
        # block (all high-only edges end up there).
        idx = np.full((P, T), -1, dtype=np.int64)
        em = np.where(edge_core == c)[0]
        d_slot0 = node_slot[dst[em]]
        pref = np.where(e_lowonly[em], 0, np.where(e_highonly[em], 2, 1))
        order = np.lexsort((pref, d_slot0))
        em = em[order]
        d_slot = d_slot0[order]
        pref = pref[order]
        grp_start = np.searchsorted(d_slot, d_slot)
        j = np.arange(len(em)) - grp_start
        w_of = d_slot // P
        p_of = d_slot % P
        rows = table_row[src[em]]
        nLowOv = np.bincount(d_slot[pref <= 1], minlength=Np)
        L = np.minimum(kA[w_of], nLowOv[d_slot])
        in_lo = j < L
        col = np.where(in_lo, offsA[w_of] + j, offsB[w_of] + (j - L))
        if split:
            assert (~in_lo | (rows < LIMIT_ROWS)).all()
            assert (in_lo | (rows >= hi_base)).all()
        assert (np.where(in_lo, j, j - L) < np.where(in_lo, kA[w_of], kB[w_of])).all()
        idx[p_of, col] = np.where(in_lo, rows, rows - hi_base)
        # sentinels: guaranteed-zero (pad) rows inside each region
        sentA = Np - 1                              # core 0 pad slot (in LO)
        sentB = n_rows - 1 - hi_base                # core C-1 pad slot, rebased
        for w in range(W):
            a0, a1 = offs[w], offs[w] + kA[w]
            b1 = offs[w + 1]
            blkA = idx[:, a0:a1]
            blkA[blkA < 0] = sentA
            blkB = idx[:, a1:b1]
            blkB[blkB < 0] = sentB
        idx = idx.astype(np.int32)

        ndst = np.ones((P, W), dtype=np.float32)
        nd = np.ones(Np, dtype=np.float32)
        nd[:n_real] = norm_dst[ns]
        ndst[:, :] = nd.reshape(W, P).T

        in_maps.append(
            {
                "xa": xa,
                "idx": idx,
                "ndst": ndst,
                "wd": wd_h,
                "wgu": wgu_h,
                "bu2": bu2_h,
            }
        )
    return in_maps, node_core, node_slot, (kA.astype(int), kB.astype(int), hi_base)


def build_graph(cfg, k_packed):
    """Build the SPMD Bass graph (same for all cores).

    """
    W = cfg.W
    kA, kB, hi_base = k_packed
    k_prof = kA + kB
    offs = np.concatenate([[0], np.cumsum(k_prof)]).astype(int)
    T = int(offs[-1])
    OUT, IN = cfg.OUT, cfg.IN

    nc = bacc.Bacc(None, target_bir_lowering=False)
    xa = nc.declare_dram_parameter("xa", [P, cfg.KIN * cfg.Np], F16, False)
    idx = nc.declare_dram_parameter("idx", [P, T], I32, False)
    ndst = nc.declare_dram_parameter("ndst", [P, W], F32, False)
    wd = nc.declare_dram_parameter("wd", [P, cfg.KIN * OUT], F16, False)
    wgu = nc.declare_dram_parameter("wgu", [OUT, IN], F16, False)
    bu2 = nc.declare_dram_parameter("bu2", [P, IN], F32, False)
    out = nc.declare_dram_parameter("out", [cfg.Np, IN], F32, True)

    with tile.TileContext(nc) as tc:
        with (
            tc.tile_pool(name="dram", bufs=1, space="DRAM") as dram,
            tc.tile_pool(name="const", bufs=1) as const,
            tc.tile_pool(name="xat", bufs=3) as xap,
            tc.tile_pool(name="hsb", bufs=3) as hsb,
            tc.tile_pool(name="dpsum", bufs=2, space="PSUM") as dpsum,
            tc.tile_pool(name="gsb", bufs=3) as gsb,
            tc.tile_pool(name="asb", bufs=2) as asb,
            tc.tile_pool(name="tpsum", bufs=2, space="PSUM") as tpsum,
            tc.tile_pool(name="atb", bufs=2) as atb,
            tc.tile_pool(name="opsum", bufs=2, space="PSUM") as opsum,
            tc.tile_pool(name="osb", bufs=2) as osb,
        ):
            h_mine = dram.tile([cfg.Np, OUT], F16)
            h_all = dram.tile([cfg.C * cfg.Np, OUT], F16)

            # persistent SBUF constants
            wd_sb = const.tile([P, cfg.KIN * OUT], F16)
            nc.sync.dma_start(out=wd_sb[:], in_=wd[:, :])
            wgu_sb = const.tile([OUT, IN], F16)
            nc.sync.dma_start(out=wgu_sb[:], in_=wgu[:, :])
            bu2_sb = const.tile([P, IN], F32)
            nc.sync.dma_start(out=bu2_sb[:], in_=bu2[:, :])
            ndst_sb = const.tile([P, W], F32)
            nc.sync.dma_start(out=ndst_sb[:], in_=ndst[:, :])
            idx_sb = const.tile([P, T], I32)
            nc.sync.dma_start(out=idx_sb[:], in_=idx[:, :])
            ident = const.tile([P, P], F32)
            make_identity(nc, ident[:])

            xa_view = xa[:, :].rearrange("p (c n) -> p c n", c=cfg.KIN)

            # ---- down-projection ----
            for nt in range(W):
                xt = xap.tile([P, cfg.KIN * P], F16)
                nc.sync.dma_start(
                    out=xt[:].rearrange("p (c n) -> p c n", c=cfg.KIN),
                    in_=xa_view[:, :, nt * P : (nt + 1) * P],
                )
                ps = dpsum.tile([P, OUT], F32, space="PSUM")
                for ccc in range(cfg.KC):
                    nc.tensor.matmul(
                        ps[:],
                        lhsT=xt[:, ccc * P : (ccc + 1) * P],
                        rhs=wd_sb[:, ccc * OUT : (ccc + 1) * OUT],
                        start=(ccc == 0),
                        stop=False,
                    )
                nc.tensor.matmul(
                    ps[:],
                    lhsT=xt[0:1, cfg.KC * P : cfg.KC * P + P],
                    rhs=wd_sb[0:1, cfg.KC * OUT : cfg.KC * OUT + OUT],
                    start=False,
                    stop=True,
                )
                ht = hsb.tile([P, OUT], F16)
                nc.vector.tensor_copy(out=ht[:], in_=ps[:])
                nc.sync.dma_start(out=h_mine[nt * P : (nt + 1) * P, :], in_=ht[:])

            # ---- all-gather h ----
            nc.gpsimd.collective_compute(
                "AllGather",
                mybir.AluOpType.bypass,
                replica_groups=[list(range(cfg.C))],
                ins=[h_mine[:].opt()],
                outs=[h_all[:].opt()],
            )


            # ---- edge aggregation + up-projection ----
            for w in range(W):
                k = int(k_prof[w])
                G = gsb.tile([P, k * OUT], F16, tag="G")
                # HW indirect DMA uses ONE offset per partition and reads the
                # out free-size contiguously -> one gather per slot column.
                # Columns < kA[w] index the low table region; the rest are
                # rebased into the high region via element_offset (the raw
                # descriptor offset field is too narrow for the full table).
                for j in range(k):
                    eo = 0 if j < int(kA[w]) else int(hi_base) * OUT
                    nc.gpsimd.indirect_dma_start(
                        out=G[:, j * OUT : (j + 1) * OUT],
                        out_offset=None,
                        in_=h_all[:],
                        in_offset=IndirectOffsetOnAxis(
                            ap=idx_sb[:, int(offs[w]) + j : int(offs[w]) + j + 1],
                            axis=0,
                        ),
                        element_offset=eo,
                    )
                agg = asb.tile([P, OUT], F32)
                if k > 1:
                    nc.vector.tensor_reduce(
                        out=agg[:],
                        in_=G[:].rearrange("p (k f) -> p f k", k=k),
                        axis=mybir.AxisListType.X,
                        op=mybir.AluOpType.add,
                    )
                else:
                    nc.vector.tensor_copy(out=agg[:], in_=G[:])
                nc.vector.tensor_scalar_mul(agg[:], agg[:], ndst_sb[:, w : w + 1])
                tps = tpsum.tile([P, OUT], F32, space="PSUM")
                nc.tensor.transpose(tps[:], agg[:], ident[:])
                aggT = atb.tile([P, OUT], F16)
                nc.vector.tensor_copy(out=aggT[:], in_=tps[:])
                ops = opsum.tile([P, IN], F32, space="PSUM")
                for lo in range(0, IN, 512):
                    hi = min(lo + 512, IN)
                    nc.tensor.matmul(
                        ops[:, lo:hi],
                        lhsT=aggT[:],
                        rhs=wgu_sb[:, lo:hi],
                        start=True,
                        stop=True,
                    )
                ot = osb.tile([P, IN], F32)
                nc.vector.tensor_tensor(
                    out=ot[:], in0=ops[:], in1=bu2_sb[:], op=mybir.AluOpType.add
                )
                nc.sync.dma_start(out=out[w * P : (w + 1) * P, :], in_=ot[:])

    nc.compile()
    return nc




_GRAPH_CACHE = {}


def kernel(features, Wd, bd, Wg, bg, Wu, bu, src, dst):
    features = np.asarray(features)
    N, IN = features.shape
    OUT = np.asarray(Wd).shape[1]
    E = np.asarray(src).shape[0]
    cfg = Cfg(N, E, IN, OUT)

    in_maps, node_core, node_slot, k_packed = host_prep(
        cfg, features, Wd, bd, Wg, bg, Wu, bu, src, dst
    )
    kA, kB, hi_base = k_packed
    key = (N, E, IN, OUT, hi_base, tuple(map(int, kA)), tuple(map(int, kB)))
    nc = _GRAPH_CACHE.get(key)
    if nc is None:
        nc = build_graph(cfg, k_packed)
        _GRAPH_CACHE[key] = nc

    res = run_bass_kernel_spmd(nc, in_maps, core_ids=list(range(cfg.C)))
    allo = np.stack([np.asarray(res.results[i]["out"]) for i in range(cfg.C)])
    return allo[node_core, node_slot, :].astype(np.float32)
